# revision 49
# baseline (speedup 1.0000x reference)
"""Multi-head attention (b=8, n=1024, dim=1024, 16 heads) on 8 TRN2 NeuronCores.

Data-parallel: one batch element per core. Each core runs an identical
Bass/Tile program computing qkv projection, softmax attention, and the
output projection for its [1024, 1024] slice, in bf16 with fp32 PSUM
accumulation.

Layout choices (host pre-transposes so the device never transposes):
  - xt   [c, n]   = x[i].T                       (bf16)
  - wqkt [c, 2h*d] = permuted q/k weights^T: head-pair p occupies
        f-tiles 2p (q rows of heads 2p,2p+1) and 2p+1 (k rows).
        A 128-row f-tile = [head 2p (64 rows); head 2p+1 (64 rows)], so
        the qkv matmul directly yields q^T/k^T pair tiles where the even
        head lives on partitions 0-63 and the odd head on 64-127.
  - wvt  [c, h*d] = wv.T, wpt [c, o] = w_proj.T  (bf16)

Per core:
  V    = x @ wv^T          -> SBUF [n, h*d]
  qk^T = wqk_perm @ x^T    -> SBUF pair tiles [128, n]
  S^T  = k_h @ q_h^T       -> PSUM [nk_tile, nq]   (K=64 row-tiled pairs:
         both heads' matmuls run CONCURRENTLY in the PE via tile_position
         row strips -- measured ~1.9x on HW)
  attn^T = exp(0.125*S^T)  -> SBUF bf16 (ScalarE; no max subtraction --
           scores ~ N(0,1), exp stays well inside fp32/bf16 range, and
           softmax is shift-invariant so the result matches jax.nn.softmax)
  out^T = V^T @ attn^T     -> PSUM [128, nq-chunk]: COL-tiled head pair,
         head even on psum partitions 0:64 (col strip 0), head odd on
         64:128 (strip 64), two interleaved accumulation chains in ONE
         bank (per-half start=True clears only that half's has_written
         bytes -- verified on HW); concurrent like the S pairs
  denom = ones^T @ attn^T  -> four M=1 chains at the 4 col strips of one
         bank (j = hh + 2*nqc -> strip row 32j), 4 concurrent MMs per
         quad-step (~282 ns), evacuated by ONE DVE copy then 4 single-lane
         reciprocals; 1/denom broadcast across 64 partitions on GPSIMD
  out^T scaled straight out of PSUM by the DVE muls (PSUM input is exempt
         from the SB same-base-partition rule, so the odd half's output
         base shift is legal)
  y    = out_heads @ w_proj^T + b   in two pieces: contraction chunks
         0-6 run during the last pair's AV phase (bias folded into the
         partial, staged bf16 in the then-dead xt SBUF region); chunk 7
         plus the partial-add form a short tail.

Software pipeline: the pair-0 S/exp prologue and every pair-p step drain
"filler" PE work units (qkv f-tiles, V tiles, proj partials) so the PE
never idles while ACT (exp) is the phase bottleneck. A few warm-up
matmuls on a zeroed tile run during the input DMA wait so the PE HAM
clock-gate is already at full rate when real work arrives.

Scheduling rules learned from HW A/B (cost model can't see them):
  - PSUM pools are partitioned by role (S 2x2-bank, fillers 2x1, AV 2x1):
    fillers sharing slots with S starves the PE during ACT-bound
    stretches (-46 us).
  - Engine queues are strict FIFO and the DVE runs ~half a pair ahead of
    the PE, so any DVE op that waits on a far-future PE event (e.g. a
    reciprocal reading the den bank) head-of-line blocks every later DVE
    op AND holds its PSUM bank hostage against PE writes (collision
    guard).  Hence: den psum is touched by exactly ONE DVE copy, placed
    at the pair start where the wait is minimal, and the reciprocals
    read the SBUF staging copy.
  - s-emit bursts need filler chains interleaved between them or the PE
    serializes against the exp drain of the two s_pool slots.

All inputs are packed into one [dim, 5120] bf16 DRAM tensor so the whole
input loads with 16 large DMAs spread over 4 DGE queues. Output is
stored bf16 (halves the store DMA) and cast to fp32 on host.  out^T
aliases the wvt SBUF region (dead after the V chains) to make room for
a 32-deep attn tile pool.
"""

import numpy as np
import ml_dtypes

B, N, DIM = 8, 1024, 1024
H, D = 16, 64
NP = 128  # partitions
NCHUNK = 512  # matmul free-dim chunk (one PSUM bank of fp32)
CT = DIM // NP  # 8 contraction chunks
NT = N // NP  # 8 n-tiles
NQC = N // NCHUNK  # 2 nq chunks
PAIRS = H // 2  # 8 head pairs

BF16 = ml_dtypes.bfloat16

_CACHE = {}


def build(loop_iters=1):
    """Build and compile the per-core Bacc graph. Cached per loop_iters."""
    if loop_iters in _CACHE:
        return _CACHE[loop_iters]

    import concourse.mybir as mybir
    import concourse.tile as tile
    from concourse import bacc

    fp32 = mybir.dt.float32
    bf16 = mybir.dt.bfloat16
    Exp = mybir.ActivationFunctionType.Exp

    nc = bacc.Bacc("TRN2", target_bir_lowering=False, debug=False, num_devices=8)

    PACK = N + 2 * DIM + DIM + DIM  # xt | wqkt | wvt | wpt along free dim
    inp = nc.declare_dram_parameter("inp", [DIM, PACK], bf16, isOutput=False)
    bias = nc.declare_dram_parameter("bias", [1, DIM], bf16, isOutput=False)
    out = nc.declare_dram_parameter("out", [N, DIM], bf16, isOutput=True)

    with tile.TileContext(nc) as tc:
        with (
            tc.tile_pool(name="weights", bufs=1) as wpool,
            tc.tile_pool(name="acts", bufs=1) as apool,
            tc.tile_pool(name="attn", bufs=32) as attnpool,
            tc.tile_pool(name="small", bufs=2) as spool,
            tc.tile_pool(name="s_ps", bufs=2, space="PSUM") as s_pool,
            tc.tile_pool(name="fill_ps", bufs=2, space="PSUM") as fill_ps,
            tc.tile_pool(name="av_ps", bufs=2, space="PSUM") as av_ps,
        ):
            # ---- persistent SBUF tensors (loaded once, one DMA per c-chunk) ----
            packed_sb = wpool.tile([NP, CT, PACK], bf16, tag="packed")

            HOT = N + 4 * NP  # xt + wqkt f-tiles of pairs 0 and 1
            dma_engines = [nc.sync, nc.scalar, nc.gpsimd]
            # one DMA per queue per region (3 c-chunks batched) -- fewer,
            # larger DMAs beat per-chunk issue+sem overhead on arrival time
            for ct in range(CT):
                eng = dma_engines[ct % 3]
                eng.dma_start(packed_sb[:, ct, 0:HOT],
                              inp[ct * NP:(ct + 1) * NP, 0:HOT])
            for ct in range(CT):
                eng = dma_engines[ct % 3]
                eng.dma_start(packed_sb[:, ct, HOT:],
                              inp[ct * NP:(ct + 1) * NP, HOT:])
            bias_bc = wpool.tile([NP, DIM], bf16, tag="biasbc")
            nc.sync.dma_start(bias_bc[:], bias[0:1, :].to_broadcast((NP, DIM)))
            xt_sb = packed_sb[:, :, 0:N]
            wqkt_sb = packed_sb[:, :, N:N + 2 * DIM]
            wvt_sb = packed_sb[:, :, N + 2 * DIM:N + 3 * DIM]
            wpt_sb = packed_sb[:, :, N + 3 * DIM:N + 4 * DIM]
            # proj partial-sum staging aliases the xt region: xt's last
            # reader (a qkv f-tile unit) finishes two pairs before the
            # partials are written, and the dep tracker orders the reuse.
            ypart = packed_sb[:, :, 0:N]
            # out^T aliases the wvt region ([128, CT, 1024] bf16, same
            # shape): wvt's last reader (V chain of n-tile 7, pair-0 g0)
            # precedes the first normalize mul that writes out^T there.
            outT_sb = packed_sb[:, :, N + 2 * DIM:N + 3 * DIM]

            # ---- HAM pre-warm: short back-to-back matmuls on a zeroed
            # tile keep the PE busy during the input-DMA wait so the
            # clock-gate releases before real matmuls start.
            warm = wpool.tile([NP, NCHUNK], bf16, tag="warm")
            nc.vector.memset(warm[:], 0.0)
            # ones column + 31 zero columns: the M=32 den matmuls then own
            # every row of their col strip (rows 1:32 get written zeros),
            # which keeps the strict interpreter's psum ownership model
            # happy when the evac copy later reads the full bank
            ones32 = wpool.tile([NP, 32], bf16, tag="ones32")
            nc.vector.memset(ones32[:], 0.0)
            nc.vector.memset(ones32[:, 0:1], 1.0)
            warm_ps = fill_ps.tile([NP, NCHUNK], fp32, tag="fill", name="warm_ps")
            for _ in range(16):
                nc.tensor.matmul(warm_ps[:, 0:NP], lhsT=warm[:, 0:NP],
                                 rhs=warm[:, 0:NP], start=True, stop=True)

            def body(_it=None):
                # ---- per-iteration SBUF ----
                q_sb = apool.tile([NP, PAIRS, N], bf16, tag="q")
                k_sb = apool.tile([NP, PAIRS, N], bf16, tag="k")
                v_sb = apool.tile([NP, NT, H * D], bf16, tag="v")

                # ---- filler units: independent PE work drained into the
                # ACT-bound stretches so the PE never starves.
                filler = []

                def drain(steps_left):
                    n = (len(filler) + steps_left - 1) // steps_left
                    for _ in range(n):
                        if filler:
                            filler.pop(0)()

                def emit_qkv_ft(p, which, nqc):
                    # one nq chunk of pair p's q^T (which=0) or k^T (which=1)
                    ft = 2 * p + which
                    dst = q_sb if which == 0 else k_sb
                    qk_ps = fill_ps.tile([NP, NCHUNK], fp32, tag="fill",
                                         name="qk_ps")
                    for ct in range(CT):
                        nc.tensor.matmul(
                            qk_ps[:],
                            lhsT=wqkt_sb[:, ct, ft * NP:(ft + 1) * NP],
                            rhs=xt_sb[:, ct, nqc * NCHUNK:(nqc + 1) * NCHUNK],
                            start=(ct == 0),
                            stop=(ct == CT - 1),
                        )
                    nc.vector.tensor_copy(
                        dst[:, p, nqc * NCHUNK:(nqc + 1) * NCHUNK], qk_ps[:])

                def emit_v(nt, fc):
                    # heads 8*fc .. 8*fc+7 of V rows nt*128..
                    v_ps = fill_ps.tile([NP, NCHUNK], fp32, tag="fill",
                                        name="v_ps")
                    for ct in range(CT):
                        nc.tensor.matmul(
                            v_ps[:],
                            lhsT=xt_sb[:, ct, nt * NP:(nt + 1) * NP],
                            rhs=wvt_sb[:, ct, fc * NCHUNK:(fc + 1) * NCHUNK],
                            start=(ct == 0),
                            stop=(ct == CT - 1),
                        )
                    nc.vector.tensor_copy(
                        v_sb[:, nt, fc * NCHUNK:(fc + 1) * NCHUNK], v_ps[:])

                def emit_proj_partial(nt):
                    # y partial: contraction chunks 0..6, bias folded in,
                    # staged bf16 into the dead xt region
                    for oc in range(NQC):
                        yp = fill_ps.tile([NP, NCHUNK], fp32, tag="fill",
                                          name="yp")
                        for ct in range(CT - 1):
                            nc.tensor.matmul(
                                yp[:],
                                lhsT=outT_sb[:, ct, nt * NP:(nt + 1) * NP],
                                rhs=wpt_sb[:, ct, oc * NCHUNK:(oc + 1) * NCHUNK],
                                start=(ct == 0),
                                stop=(ct == CT - 2),
                            )
                        nc.vector.tensor_add(
                            ypart[:, nt, oc * NCHUNK:(oc + 1) * NCHUNK], yp[:],
                            bias_bc[:, oc * NCHUNK:(oc + 1) * NCHUNK],
                        )

                def emit_s_exp(p, nkt, atn):
                    # S^T for both heads of pair p at nk-tile nkt; even head
                    # on PE rows 0-63, odd on 64-127 (row-tiled, concurrent)
                    sps = {}
                    for hh in range(2):
                        sps[hh] = s_pool.tile([NP, 2 * NCHUNK], fp32,
                                              tag="s", name="s_ps")
                    for nqc in range(NQC):
                        for hh in range(2):
                            lo, hi = hh * D, (hh + 1) * D
                            nc.tensor.matmul(
                                sps[hh][:, nqc * NCHUNK:(nqc + 1) * NCHUNK],
                                lhsT=k_sb[lo:hi, p, nkt * NP:(nkt + 1) * NP],
                                rhs=q_sb[lo:hi, p, nqc * NCHUNK:(nqc + 1) * NCHUNK],
                                start=True,
                                stop=True,
                                tile_position=(hh * D, 0),
                            )
                    for hh in range(2):
                        a = attnpool.tile([NP, N], bf16, tag="attn")
                        nc.scalar.activation(a[:], sps[hh][:], Exp,
                                             scale=float(D) ** -0.5)
                        atn[hh, nkt] = a

                def emit_den(p):
                    # softmax denominators for pair p, all nq: four M=1
                    # ones-matmul chains in the 4 col-strips of one bank,
                    # j = hh + 2*nqc -> strip row 32j.  Chains interleave in
                    # one accumulation group per strip (per-strip start=True
                    # clears only that strip's has_written bytes -- verified
                    # on HW).  Reciprocals evacuate eagerly to free the bank.
                    den = fill_ps.tile([NP, NCHUNK], fp32, tag="fill",
                                       name="den")
                    for nkt in range(NT):
                        for j in range(4):
                            hh, qc = j & 1, j >> 1
                            nc.tensor.matmul(
                                den[32 * j:32 * (j + 1), :],
                                lhsT=ones32[:],
                                rhs=atn_cur[hh, nkt][
                                    :, qc * NCHUNK:(qc + 1) * NCHUNK],
                                start=(nkt == 0),
                                stop=(nkt == NT - 1),
                                tile_position=(0, 32 * j),
                                skip_group_check=True,
                            )
                    # ACT evacuates the bank: a pending DVE read would hold
                    # the fill_ps bank hostage (PE-write collision guard)
                    # until the far-behind DVE queue reaches it; ACT's queue
                    # at the pair boundary is short and ScE reads PSUM fast
                    den_sb = spool.tile([NP, NCHUNK], bf16, tag="densb",
                                        bufs=2, name="den_sb")
                    nc.vector.tensor_copy(den_sb[:], den[:])
                    return den_sb

                def emit_recips(den):
                    # emitted AFTER the g0 drain copies: DVE is strict FIFO
                    # and runs far ahead of the PE, so a reciprocal waiting
                    # on the den chain head-of-line blocks every DVE op
                    # emitted after it -- keep only the muls (which need the
                    # recips anyway) behind this point
                    recips = []
                    for j in range(4):
                        rc = spool.tile([1, NCHUNK], bf16, tag="recip",
                                        bufs=4, name="recip")
                        with nc.allow_low_precision(
                                reason="softmax 1/den in bf16; output gate "
                                       "is 2e-2 relative error"):
                            nc.vector.reciprocal(rc[:],
                                                 den[32 * j:32 * j + 1, :])
                        recips.append(rc)
                    return recips

                def emit_av_chain(p, nqc):
                    # col-paired softmax-weighted value accumulation: head
                    # even on PSUM partitions 0:64 (col strip 0), head odd on
                    # 64:128 (strip 64), two concurrent interleaved chains in
                    # one bank
                    avt = av_ps.tile([NP, NCHUNK], fp32, tag="av", name="av")
                    for nkt in range(NT):
                        for hh in range(2):
                            h = 2 * p + hh
                            nc.tensor.matmul(
                                avt[hh * D:(hh + 1) * D, :],
                                lhsT=v_sb[:, nkt, h * D:(h + 1) * D],
                                rhs=atn_cur[hh, nkt][
                                    :, nqc * NCHUNK:(nqc + 1) * NCHUNK],
                                start=(nkt == 0),
                                stop=(nkt == NT - 1),
                                tile_position=(0, hh * D),
                                skip_group_check=True,
                            )
                    return avt

                def normalize(p, nqc, avt, recips):
                    # scale straight out of PSUM: one input is PSUM, so the
                    # SB same-base-partition rule doesn't bind and the DVE
                    # shifts the output base per half.
                    dst = outT_sb[:, p, nqc * NCHUNK:(nqc + 1) * NCHUNK]
                    for hh in range(2):
                        rb = spool.tile([D, NCHUNK], bf16, tag="recipb",
                                        bufs=2, name="rb")
                        nc.gpsimd.partition_broadcast(
                            rb[:], recips[hh + 2 * nqc][:], channels=D)
                        nc.vector.tensor_mul(
                            dst[hh * D:(hh + 1) * D, :],
                            avt[hh * D:(hh + 1) * D, :], rb[:])

                def emit_y_tail(nt, eng):
                    y_sb = spool.tile([NP, N], bf16, tag="ysb", bufs=2,
                                      name="y_sb")
                    for oc in range(NQC):
                        yp2 = fill_ps.tile([NP, NCHUNK], fp32, tag="fill",
                                           name="yp2")
                        nc.tensor.matmul(
                            yp2[:],
                            lhsT=outT_sb[:, CT - 1, nt * NP:(nt + 1) * NP],
                            rhs=wpt_sb[:, CT - 1, oc * NCHUNK:(oc + 1) * NCHUNK],
                            start=True,
                            stop=True,
                        )
                        nc.vector.tensor_add(
                            y_sb[:, oc * NCHUNK:(oc + 1) * NCHUNK], yp2[:],
                            ypart[:, nt, oc * NCHUNK:(oc + 1) * NCHUNK],
                        )
                    eng.dma_start(out[nt * NP:(nt + 1) * NP, :], y_sb[:])

                # ---- schedule ----
                # qkv(0) immediately; the S(0)/exp prologue drains qkv(1)
                # and all V tiles as filler (PE-bound is fine -- ACT has
                # slack); each pair is four AV groups, each preceded by
                # two S(p+1) emissions and filler.
                for which in range(2):
                    for nqc in range(NQC):
                        emit_qkv_ft(0, which, nqc)
                for which in range(2):
                    for nqc in range(NQC):
                        filler.append(
                            lambda w=which, q=nqc: emit_qkv_ft(1, w, q))
                for nt in range(4):
                    for fc in range(2):
                        filler.append(lambda n=nt, f=fc: emit_v(n, f))

                atn_cur = {}
                for nkt in range(NT):
                    emit_s_exp(0, nkt, atn_cur)
                    drain(NT - nkt)

                for p in range(PAIRS):
                    if p + 2 < PAIRS:
                        for which in range(2):
                            for nqc in range(NQC):
                                filler.append(
                                    lambda w=which, q=nqc, pp=p + 2:
                                    emit_qkv_ft(pp, w, q))
                    last = p == PAIRS - 1
                    if last:
                        for nt in range(NT):
                            filler.append(lambda n=nt: emit_proj_partial(n))

                    atn_nxt = {}
                    snkt = 0
                    avts = {}
                    den_sb = emit_den(p)
                    recips = None
                    for g in range(NQC):  # two nqc-major AV groups per pair
                        if p + 1 < PAIRS:
                            for i in range(4):
                                emit_s_exp(p + 1, snkt, atn_nxt)
                                snkt += 1
                                if i == 1 and not last:
                                    # filler between s-emit pairs: the PE
                                    # queue is strict FIFO, so without this
                                    # the 4-emit burst serializes PE against
                                    # the exp drain of the s_pool slots
                                    drain(4 - 2 * g)
                        if p == 0 and g == 0:
                            # V tiles 4-7 land just before their first AV
                            # reads; keeping them out of the prologue evens
                            # the PE pressure against the exp pace there
                            for nt in range(4, NT):
                                for fc in range(2):
                                    emit_v(nt, fc)
                        if not last:
                            drain(3 - 2 * g)
                        avts[g] = emit_av_chain(p, g)
                        if g == 0:
                            recips = emit_recips(den_sb)
                        normalize(p, g, avts[g], recips)
                        if last:
                            drain(2 - g)
                            for nt in range(4 * g, 4 * g + 4):
                                emit_y_tail(nt, dma_engines[nt % 3])
                    atn_cur = atn_nxt

            if loop_iters == 1:
                body()
            else:
                with tc.For_i(0, loop_iters, 1) as it:
                    body(it)

    nc.compile()
    _CACHE[loop_iters] = nc
    return nc


def prep_inputs(x, w_qkv, w_proj, b_proj):
    """Host-side sharding + layout prep -> per-core input maps."""
    wq, wk, wv = w_qkv[0:DIM], w_qkv[DIM:2 * DIM], w_qkv[2 * DIM:3 * DIM]
    perm = []
    for p in range(PAIRS):
        perm.append(wq[2 * p * D:(2 * p + 2) * D])
        perm.append(wk[2 * p * D:(2 * p + 2) * D])
    wqk_perm = np.concatenate(perm, axis=0)  # [2*DIM, DIM]
    w_cols = np.concatenate([wqk_perm.T, wv.T, w_proj.T], axis=1).astype(BF16)
    bias = b_proj.reshape(1, DIM).astype(BF16)
    in_maps = []
    for i in range(B):
        xt = x[i].T.astype(BF16)
        inp = np.ascontiguousarray(np.concatenate([xt, w_cols], axis=1))
        in_maps.append({"inp": inp, "bias": bias})
    return in_maps


def kernel(x, w_qkv, w_proj, b_proj):
    from concourse import bass_utils

    x = np.asarray(x, dtype=np.float32)
    w_qkv = np.asarray(w_qkv, dtype=np.float32)
    w_proj = np.asarray(w_proj, dtype=np.float32)
    b_proj = np.asarray(b_proj, dtype=np.float32)
    assert x.shape == (B, N, DIM)

    nc = build(1)
    in_maps = prep_inputs(x, w_qkv, w_proj, b_proj)
    res = bass_utils.run_bass_kernel_spmd(nc, in_maps, core_ids=list(range(B)))
    return np.stack(
        [res.results[i]["out"].astype(np.float32) for i in range(B)], axis=0)



# revision 51
# speedup vs baseline: 1.0000x; 1.0000x over previous
"""Multi-head attention (b=8, n=1024, dim=1024, 16 heads) on 8 TRN2 NeuronCores.

Data-parallel: one batch element per core. Each core runs an identical
Bass/Tile program computing qkv projection, softmax attention, and the
output projection for its [1024, 1024] slice, in bf16 with fp32 PSUM
accumulation.

Layout choices (host pre-transposes so the device never transposes):
  - xt   [c, n]   = x[i].T                       (bf16)
  - wqkt [c, 2h*d] = permuted q/k weights^T: head-pair p occupies
        f-tiles 2p (q rows of heads 2p,2p+1) and 2p+1 (k rows).
        A 128-row f-tile = [head 2p (64 rows); head 2p+1 (64 rows)], so
        the qkv matmul directly yields q^T/k^T pair tiles where the even
        head lives on partitions 0-63 and the odd head on 64-127.
  - wvt  [c, h*d] = wv.T, wpt [c, o] = w_proj.T  (bf16)

Per core:
  V    = x @ wv^T          -> SBUF [n, h*d]
  qk^T = wqk_perm @ x^T    -> SBUF pair tiles [128, n]
  S^T  = k_h @ q_h^T       -> PSUM [nk_tile, nq]   (K=64 row-tiled pairs:
         both heads' matmuls run CONCURRENTLY in the PE via tile_position
         row strips -- measured ~1.9x on HW)
  attn^T = exp(0.125*S^T)  -> SBUF bf16 (ScalarE; no max subtraction --
           scores ~ N(0,1), exp stays well inside fp32/bf16 range, and
           softmax is shift-invariant so the result matches jax.nn.softmax)
  out^T = V^T @ attn^T     -> PSUM [128, nq-chunk]: COL-tiled head pair,
         head even on psum partitions 0:64 (col strip 0), head odd on
         64:128 (strip 64), two interleaved accumulation chains in ONE
         bank (per-half start=True clears only that half's has_written
         bytes -- verified on HW); concurrent like the S pairs
  denom = ones^T @ attn^T  -> four M=1 chains at the 4 col strips of one
         bank (j = hh + 2*nqc -> strip row 32j), 4 concurrent MMs per
         quad-step (~282 ns), evacuated by ONE DVE copy then 4 single-lane
         reciprocals; 1/denom broadcast across 64 partitions on GPSIMD
  out^T scaled straight out of PSUM by the DVE muls (PSUM input is exempt
         from the SB same-base-partition rule, so the odd half's output
         base shift is legal)
  y    = out_heads @ w_proj^T + b   in two pieces: contraction chunks
         0-6 run during the last pair's AV phase (bias folded into the
         partial, staged bf16 in the then-dead xt SBUF region); chunk 7
         plus the partial-add form a short tail.

Software pipeline: the pair-0 S/exp prologue and every pair-p step drain
"filler" PE work units (qkv f-tiles, V tiles, proj partials) so the PE
never idles while ACT (exp) is the phase bottleneck. A few warm-up
matmuls on a zeroed tile run during the input DMA wait so the PE HAM
clock-gate is already at full rate when real work arrives.

Scheduling rules learned from HW A/B (cost model can't see them):
  - PSUM pools are partitioned by role (S 2x2-bank, fillers 2x1, AV 2x1):
    fillers sharing slots with S starves the PE during ACT-bound
    stretches (-46 us).
  - Engine queues are strict FIFO and the DVE runs ~half a pair ahead of
    the PE, so any DVE op that waits on a far-future PE event (e.g. a
    reciprocal reading the den bank) head-of-line blocks every later DVE
    op AND holds its PSUM bank hostage against PE writes (collision
    guard).  Hence: den psum is touched by exactly ONE DVE copy, placed
    at the pair start where the wait is minimal, and the reciprocals
    read the SBUF staging copy.
  - s-emit bursts need filler chains interleaved between them or the PE
    serializes against the exp drain of the two s_pool slots.

All inputs are packed into one [dim, 5120] bf16 DRAM tensor so the whole
input loads with 16 large DMAs spread over 4 DGE queues. Output is
stored bf16 (halves the store DMA) and cast to fp32 on host.  out^T
aliases the wvt SBUF region (dead after the V chains) to make room for
a 32-deep attn tile pool.
"""

import numpy as np
import ml_dtypes

B, N, DIM = 8, 1024, 1024
H, D = 16, 64
NP = 128  # partitions
NCHUNK = 512  # matmul free-dim chunk (one PSUM bank of fp32)
CT = DIM // NP  # 8 contraction chunks
NT = N // NP  # 8 n-tiles
NQC = N // NCHUNK  # 2 nq chunks
PAIRS = H // 2  # 8 head pairs

BF16 = ml_dtypes.bfloat16

_CACHE = {}


def build(loop_iters=1):
    """Build and compile the per-core Bacc graph. Cached per loop_iters."""
    if loop_iters in _CACHE:
        return _CACHE[loop_iters]

    import concourse.mybir as mybir
    import concourse.tile as tile
    from concourse import bacc

    fp32 = mybir.dt.float32
    bf16 = mybir.dt.bfloat16
    Exp = mybir.ActivationFunctionType.Exp

    nc = bacc.Bacc("TRN2", target_bir_lowering=False, debug=False, num_devices=8)

    PACK = N + 2 * DIM + DIM + DIM  # xt | wqkt | wvt | wpt along free dim
    inp = nc.declare_dram_parameter("inp", [DIM, PACK], bf16, isOutput=False)
    bias = nc.declare_dram_parameter("bias", [1, DIM], bf16, isOutput=False)
    out = nc.declare_dram_parameter("out", [N, DIM], bf16, isOutput=True)

    with tile.TileContext(nc) as tc:
        with (
            tc.tile_pool(name="weights", bufs=1) as wpool,
            tc.tile_pool(name="acts", bufs=1) as apool,
            tc.tile_pool(name="attn", bufs=32) as attnpool,
            tc.tile_pool(name="small", bufs=2) as spool,
            tc.tile_pool(name="s_ps", bufs=2, space="PSUM") as s_pool,
            tc.tile_pool(name="fill_ps", bufs=2, space="PSUM") as fill_ps,
            tc.tile_pool(name="av_ps", bufs=2, space="PSUM") as av_ps,
        ):
            # ---- persistent SBUF tensors (loaded once, one DMA per c-chunk) ----
            packed_sb = wpool.tile([NP, CT, PACK], bf16, tag="packed")

            HOT = N + 4 * NP  # xt + wqkt f-tiles of pairs 0 and 1
            dma_engines = [nc.sync, nc.scalar, nc.gpsimd]
            # one DMA per queue per region (3 c-chunks batched) -- fewer,
            # larger DMAs beat per-chunk issue+sem overhead on arrival time
            for ct in range(CT):
                eng = dma_engines[ct % 3]
                eng.dma_start(packed_sb[:, ct, 0:HOT],
                              inp[ct * NP:(ct + 1) * NP, 0:HOT])
            for ct in range(CT):
                eng = dma_engines[ct % 3]
                eng.dma_start(packed_sb[:, ct, HOT:],
                              inp[ct * NP:(ct + 1) * NP, HOT:])
            bias_bc = wpool.tile([NP, DIM], bf16, tag="biasbc")
            nc.sync.dma_start(bias_bc[:], bias[0:1, :].to_broadcast((NP, DIM)))
            xt_sb = packed_sb[:, :, 0:N]
            wqkt_sb = packed_sb[:, :, N:N + 2 * DIM]
            wvt_sb = packed_sb[:, :, N + 2 * DIM:N + 3 * DIM]
            wpt_sb = packed_sb[:, :, N + 3 * DIM:N + 4 * DIM]
            # proj partial-sum staging aliases the xt region: xt's last
            # reader (a qkv f-tile unit) finishes two pairs before the
            # partials are written, and the dep tracker orders the reuse.
            ypart = packed_sb[:, :, 0:N]
            # out^T aliases the wvt region ([128, CT, 1024] bf16, same
            # shape): wvt's last reader (V chain of n-tile 7, pair-0 g0)
            # precedes the first normalize mul that writes out^T there.
            outT_sb = packed_sb[:, :, N + 2 * DIM:N + 3 * DIM]

            # ---- HAM pre-warm: short back-to-back matmuls on a zeroed
            # tile keep the PE busy during the input-DMA wait so the
            # clock-gate releases before real matmuls start.
            warm = wpool.tile([NP, NCHUNK], bf16, tag="warm")
            nc.vector.memset(warm[:], 0.0)
            # ones column + 31 zero columns: the M=32 den matmuls then own
            # every row of their col strip (rows 1:32 get written zeros),
            # which keeps the strict interpreter's psum ownership model
            # happy when the evac copy later reads the full bank
            ones32 = wpool.tile([NP, 32], bf16, tag="ones32")
            nc.vector.memset(ones32[:], 0.0)
            nc.vector.memset(ones32[:, 0:1], 1.0)
            warm_ps = fill_ps.tile([NP, NCHUNK], fp32, tag="fill", name="warm_ps")
            for _ in range(16):
                nc.tensor.matmul(warm_ps[:, 0:NP], lhsT=warm[:, 0:NP],
                                 rhs=warm[:, 0:NP], start=True, stop=True)

            def body(_it=None):
                # ---- per-iteration SBUF ----
                q_sb = apool.tile([NP, PAIRS, N], bf16, tag="q")
                k_sb = apool.tile([NP, PAIRS, N], bf16, tag="k")
                v_sb = apool.tile([NP, NT, H * D], bf16, tag="v")

                # ---- filler units: independent PE work drained into the
                # ACT-bound stretches so the PE never starves.
                filler = []

                def drain(steps_left):
                    n = (len(filler) + steps_left - 1) // steps_left
                    for _ in range(n):
                        if filler:
                            filler.pop(0)()

                def emit_qkv_ft(p, which, nqc):
                    # one nq chunk of pair p's q^T (which=0) or k^T (which=1)
                    ft = 2 * p + which
                    dst = q_sb if which == 0 else k_sb
                    qk_ps = fill_ps.tile([NP, NCHUNK], fp32, tag="fill",
                                         name="qk_ps")
                    for ct in range(CT):
                        nc.tensor.matmul(
                            qk_ps[:],
                            lhsT=wqkt_sb[:, ct, ft * NP:(ft + 1) * NP],
                            rhs=xt_sb[:, ct, nqc * NCHUNK:(nqc + 1) * NCHUNK],
                            start=(ct == 0),
                            stop=(ct == CT - 1),
                        )
                    nc.vector.tensor_copy(
                        dst[:, p, nqc * NCHUNK:(nqc + 1) * NCHUNK], qk_ps[:])

                def emit_v(nt, fc):
                    # heads 8*fc .. 8*fc+7 of V rows nt*128..
                    v_ps = fill_ps.tile([NP, NCHUNK], fp32, tag="fill",
                                        name="v_ps")
                    for ct in range(CT):
                        nc.tensor.matmul(
                            v_ps[:],
                            lhsT=xt_sb[:, ct, nt * NP:(nt + 1) * NP],
                            rhs=wvt_sb[:, ct, fc * NCHUNK:(fc + 1) * NCHUNK],
                            start=(ct == 0),
                            stop=(ct == CT - 1),
                        )
                    nc.vector.tensor_copy(
                        v_sb[:, nt, fc * NCHUNK:(fc + 1) * NCHUNK], v_ps[:])

                def emit_proj_partial(nt):
                    # y partial: contraction chunks 0..6, bias folded in,
                    # staged bf16 into the dead xt region
                    for oc in range(NQC):
                        yp = fill_ps.tile([NP, NCHUNK], fp32, tag="fill",
                                          name="yp")
                        for ct in range(CT - 1):
                            nc.tensor.matmul(
                                yp[:],
                                lhsT=outT_sb[:, ct, nt * NP:(nt + 1) * NP],
                                rhs=wpt_sb[:, ct, oc * NCHUNK:(oc + 1) * NCHUNK],
                                start=(ct == 0),
                                stop=(ct == CT - 2),
                            )
                        nc.vector.tensor_add(
                            ypart[:, nt, oc * NCHUNK:(oc + 1) * NCHUNK], yp[:],
                            bias_bc[:, oc * NCHUNK:(oc + 1) * NCHUNK],
                        )

                def emit_s_exp(p, nkt, atn):
                    # S^T for both heads of pair p at nk-tile nkt; even head
                    # on PE rows 0-63, odd on 64-127 (row-tiled, concurrent)
                    sps = {}
                    for hh in range(2):
                        sps[hh] = s_pool.tile([NP, 2 * NCHUNK], fp32,
                                              tag="s", name="s_ps")
                    for nqc in range(NQC):
                        for hh in range(2):
                            lo, hi = hh * D, (hh + 1) * D
                            nc.tensor.matmul(
                                sps[hh][:, nqc * NCHUNK:(nqc + 1) * NCHUNK],
                                lhsT=k_sb[lo:hi, p, nkt * NP:(nkt + 1) * NP],
                                rhs=q_sb[lo:hi, p, nqc * NCHUNK:(nqc + 1) * NCHUNK],
                                start=True,
                                stop=True,
                                tile_position=(hh * D, 0),
                            )
                    for hh in range(2):
                        a = attnpool.tile([NP, N], bf16, tag="attn")
                        nc.scalar.activation(a[:], sps[hh][:], Exp,
                                             scale=float(D) ** -0.5)
                        atn[hh, nkt] = a

                def emit_den(p):
                    # softmax denominators for pair p, all nq: four M=1
                    # ones-matmul chains in the 4 col-strips of one bank,
                    # j = hh + 2*nqc -> strip row 32j.  Chains interleave in
                    # one accumulation group per strip (per-strip start=True
                    # clears only that strip's has_written bytes -- verified
                    # on HW).  Reciprocals evacuate eagerly to free the bank.
                    den = fill_ps.tile([NP, NCHUNK], fp32, tag="fill",
                                       name="den")
                    for nkt in range(NT):
                        for j in range(4):
                            hh, qc = j & 1, j >> 1
                            nc.tensor.matmul(
                                den[32 * j:32 * (j + 1), :],
                                lhsT=ones32[:],
                                rhs=atn_cur[hh, nkt][
                                    :, qc * NCHUNK:(qc + 1) * NCHUNK],
                                start=(nkt == 0),
                                stop=(nkt == NT - 1),
                                tile_position=(0, 32 * j),
                                skip_group_check=True,
                            )
                    # ACT evacuates the bank: a pending DVE read would hold
                    # the fill_ps bank hostage (PE-write collision guard)
                    # until the far-behind DVE queue reaches it; ACT's queue
                    # at the pair boundary is short and ScE reads PSUM fast
                    den_sb = spool.tile([NP, NCHUNK], bf16, tag="densb",
                                        bufs=2, name="den_sb")
                    nc.vector.tensor_copy(den_sb[:], den[:])
                    return den_sb

                def emit_recips(den):
                    # emitted AFTER the g0 drain copies: DVE is strict FIFO
                    # and runs far ahead of the PE, so a reciprocal waiting
                    # on the den chain head-of-line blocks every DVE op
                    # emitted after it -- keep only the muls (which need the
                    # recips anyway) behind this point
                    recips = []
                    for j in range(4):
                        rc = spool.tile([1, NCHUNK], bf16, tag="recip",
                                        bufs=4, name="recip")
                        with nc.allow_low_precision(
                                reason="softmax 1/den in bf16; output gate "
                                       "is 2e-2 relative error"):
                            nc.vector.reciprocal(rc[:],
                                                 den[32 * j:32 * j + 1, :])
                        recips.append(rc)
                    return recips

                def emit_av_chain(p, nqc):
                    # col-paired softmax-weighted value accumulation: head
                    # even on PSUM partitions 0:64 (col strip 0), head odd on
                    # 64:128 (strip 64), two concurrent interleaved chains in
                    # one bank
                    avt = av_ps.tile([NP, NCHUNK], fp32, tag="av", name="av")
                    for nkt in range(NT):
                        for hh in range(2):
                            h = 2 * p + hh
                            nc.tensor.matmul(
                                avt[hh * D:(hh + 1) * D, :],
                                lhsT=v_sb[:, nkt, h * D:(h + 1) * D],
                                rhs=atn_cur[hh, nkt][
                                    :, nqc * NCHUNK:(nqc + 1) * NCHUNK],
                                start=(nkt == 0),
                                stop=(nkt == NT - 1),
                                tile_position=(0, hh * D),
                                skip_group_check=True,
                            )
                    return avt

                def normalize(p, nqc, avt, recips):
                    # scale straight out of PSUM: one input is PSUM, so the
                    # SB same-base-partition rule doesn't bind and the DVE
                    # shifts the output base per half.
                    dst = outT_sb[:, p, nqc * NCHUNK:(nqc + 1) * NCHUNK]
                    for hh in range(2):
                        rb = spool.tile([D, NCHUNK], bf16, tag="recipb",
                                        bufs=2, name="rb")
                        nc.gpsimd.partition_broadcast(
                            rb[:], recips[hh + 2 * nqc][:], channels=D)
                        nc.vector.tensor_mul(
                            dst[hh * D:(hh + 1) * D, :],
                            avt[hh * D:(hh + 1) * D, :], rb[:])

                def emit_y_tail(nt, eng):
                    y_sb = spool.tile([NP, N], bf16, tag="ysb", bufs=2,
                                      name="y_sb")
                    for oc in range(NQC):
                        yp2 = fill_ps.tile([NP, NCHUNK], fp32, tag="fill",
                                           name="yp2")
                        nc.tensor.matmul(
                            yp2[:],
                            lhsT=outT_sb[:, CT - 1, nt * NP:(nt + 1) * NP],
                            rhs=wpt_sb[:, CT - 1, oc * NCHUNK:(oc + 1) * NCHUNK],
                            start=True,
                            stop=True,
                        )
                        nc.vector.tensor_add(
                            y_sb[:, oc * NCHUNK:(oc + 1) * NCHUNK], yp2[:],
                            ypart[:, nt, oc * NCHUNK:(oc + 1) * NCHUNK],
                        )
                    eng.dma_start(out[nt * NP:(nt + 1) * NP, :], y_sb[:])

                # ---- schedule ----
                # qkv(0) immediately; the S(0)/exp prologue drains qkv(1)
                # and all V tiles as filler (PE-bound is fine -- ACT has
                # slack); each pair is four AV groups, each preceded by
                # two S(p+1) emissions and filler.
                for which in range(2):
                    for nqc in range(NQC):
                        emit_qkv_ft(0, which, nqc)
                for which in range(2):
                    for nqc in range(NQC):
                        filler.append(
                            lambda w=which, q=nqc: emit_qkv_ft(1, w, q))
                for nt in range(4):
                    for fc in range(2):
                        filler.append(lambda n=nt, f=fc: emit_v(n, f))

                atn_cur = {}
                for nkt in range(NT):
                    emit_s_exp(0, nkt, atn_cur)
                    drain(NT - nkt)

                for p in range(PAIRS):
                    if p + 2 < PAIRS:
                        for which in range(2):
                            for nqc in range(NQC):
                                filler.append(
                                    lambda w=which, q=nqc, pp=p + 2:
                                    emit_qkv_ft(pp, w, q))
                    last = p == PAIRS - 1
                    if last:
                        for nt in range(NT):
                            filler.append(lambda n=nt: emit_proj_partial(n))

                    atn_nxt = {}
                    snkt = 0
                    avts = {}
                    den_sb = emit_den(p)
                    recips = None
                    for g in range(NQC):  # two nqc-major AV groups per pair
                        if p + 1 < PAIRS:
                            for i in range(4):
                                emit_s_exp(p + 1, snkt, atn_nxt)
                                snkt += 1
                                if i == 1 and not last:
                                    # filler between s-emit pairs: the PE
                                    # queue is strict FIFO, so without this
                                    # the 4-emit burst serializes PE against
                                    # the exp drain of the s_pool slots
                                    drain(4 - 2 * g)
                        if p == 0 and g == 0:
                            # V tiles 4-7 land just before their first AV
                            # reads; keeping them out of the prologue evens
                            # the PE pressure against the exp pace there
                            for nt in range(4, NT):
                                for fc in range(2):
                                    emit_v(nt, fc)
                        if not last:
                            drain(3 - 2 * g)
                        avts[g] = emit_av_chain(p, g)
                        if g == 0:
                            recips = emit_recips(den_sb)
                        normalize(p, g, avts[g], recips)
                        if last:
                            drain(2 - g)
                            for nt in range(4 * g, 4 * g + 4):
                                emit_y_tail(nt, dma_engines[nt % 3])
                    atn_cur = atn_nxt

            if loop_iters == 1:
                body()
            else:
                with tc.For_i(0, loop_iters, 1) as it:
                    body(it)

    nc.compile()
    _CACHE[loop_iters] = nc
    return nc


def prep_inputs(x, w_qkv, w_proj, b_proj):
    """Host-side sharding + layout prep -> per-core input maps."""
    wq, wk, wv = w_qkv[0:DIM], w_qkv[DIM:2 * DIM], w_qkv[2 * DIM:3 * DIM]
    perm = []
    for p in range(PAIRS):
        perm.append(wq[2 * p * D:(2 * p + 2) * D])
        perm.append(wk[2 * p * D:(2 * p + 2) * D])
    wqk_perm = np.concatenate(perm, axis=0)  # [2*DIM, DIM]
    w_cols = np.concatenate([wqk_perm.T, wv.T, w_proj.T], axis=1).astype(BF16)
    bias = b_proj.reshape(1, DIM).astype(BF16)
    in_maps = []
    for i in range(B):
        xt = x[i].T.astype(BF16)
        inp = np.ascontiguousarray(np.concatenate([xt, w_cols], axis=1))
        in_maps.append({"inp": inp, "bias": bias})
    return in_maps


def kernel(x, w_qkv, w_proj, b_proj):
    from concourse import bass_utils

    x = np.asarray(x, dtype=np.float32)
    w_qkv = np.asarray(w_qkv, dtype=np.float32)
    w_proj = np.asarray(w_proj, dtype=np.float32)
    b_proj = np.asarray(b_proj, dtype=np.float32)
    assert x.shape == (B, N, DIM)

    nc = build(1)
    in_maps = prep_inputs(x, w_qkv, w_proj, b_proj)
    res = bass_utils.run_bass_kernel_spmd(nc, in_maps, core_ids=list(range(B)))
    return np.stack(
        [res.results[i]["out"].astype(np.float32) for i in range(B)], axis=0)

